# revision 1
# baseline (speedup 1.0000x reference)
"""MoE routed expert matmul on 8 Trainium2 NeuronCores.

Problem: out[n] = input[n] @ w[inds[n]] + b[inds[n]]
  input [262144, 32] f32, inds [262144] i32 (1024 experts), w [1024, 32, 32], b [1024, 1, 32]

Strategy (expert-sharded; host does routing/layout only — all FLOPs on device):
  * Host sorts tokens by expert (argsort) and packs, per core k (owning experts
    128k..128k+127), a zero-padded transposed activation tensor (features on
    partitions).  Experts sit in 8 groups of 16 = 4 row-blocks r x 4 column
    slots c; each expert slot is padded to Pc[g, r, c] = max token count over
    the 8 cores (exact, per slot) so the SPMD program is identical on every
    core while padding stays ~8%; quad PSUM/output width P[g, r] = max_c Pc.
  * Matmul dtype modes:
      float16 (default): x and w are fp16 (well within range for this data;
        ~5e-4 max scale-relative error with fp32 PSUM accumulation) and run
        16 concurrent [K=32, M=32, N=P] matmuls per group via tile_position
        row+col tiling; the 4 experts of quad (g, r) stack on PSUM partitions
        32c of one bank, so one [128, P] Vector/Scalar op does the
        PSUM->SBUF copy + per-expert bias at full lane utilization.
        bfloat16 works identically (~8x larger error).
      float32r: full-rate fp32 storage mode, but its PSUM destination must be
        partition 0 (no column tiling): each expert gets its own PSUM bank
        region and quad copies run at 32-partition width (4x the op cost).
  * Loads issue on the SP HWDGE ring; first-half stores on the GpSimd SWDGE
    ring (semaphore waits park on the otherwise-idle Pool sequencer, and its
    descriptor generation runs off the critical path), second-half stores on
    the Scalar HWDGE ring after ACT's last op of the group.  The wp load is
    split so group 0's weights land first, and group 0's xt load is row-split
    so quad (0, 0) starts as early as possible.
  * Host scatters the sorted outputs back to the original token order.

Layouts (core k, local expert e_local = 16g + 4r + c, g<8, r<4, c<4):
  xt [128, TOTW]  xt[32r+i, X[g] + xoff[g,r,c] + t] = x[token t, feat i]
  wp [128, 1024]  wp[32r+i, (4g+c)*32+o]         = w[e, i, o]
  bf16 mode:
    bp [128, 32]   bp[32c+o, 4g+r]               = b[e, 0, o]
    ot [128, TOTP] ot[32c+o, Y[g,r] + t]         = out[token t of expert, feat o]
  float32r mode:
    bp [32, 128]   bp[o, e_local]                = b[e, 0, o]
    ot [128, TOTW] ot[32r+o, X[g] + c*P[g,r]+t]  = out[token t of expert, feat o]
"""

import numpy as np

import concourse.bass as bass
import concourse.mybir as mybir
import concourse.tile as tile
from concourse import bacc
from concourse.bass_utils import run_bass_kernel_spmd

N_TOK = 262144
E = 1024
F = 32
O = 32
NCORES = 8
E_LOCAL = E // NCORES  # 128 experts per core
GROUPS = E_LOCAL // 16  # 8 groups of 16 experts
F32 = mybir.dt.float32

MM_DT = mybir.dt.float16  # matmul operand dtype: float16 | bfloat16 | float32r
OT_DT = mybir.dt.float32  # output DRAM dtype (f32 for accuracy margin)

ACT_QUADS = 12  # (fp32r mode) how many of the 32 quads copy out via ScalarE

_programs: dict[tuple, "bacc.Bacc"] = {}


class _CapacityOverflow(Exception):
    """A single expert got >512 tokens (never happens for uniform routing at
    N/E = 256 tokens/expert; 512 is ~16 sigma out).  Handled by a host
    fallback so kernel() still returns a correct result."""


def _plan(counts):
    """Per-slot capacities Pc[g, r, c], quad widths P[g, r], layout offsets."""
    c4 = counts.reshape(NCORES, GROUPS, 4, 4)  # [k, g, r, c]
    if MM_DT == mybir.dt.float32r:
        # fp32r is 4x slower below N=256; keep quad-uniform padded slots
        Pq = np.maximum(256, -(-c4.max(axis=(0, 3)) // 16) * 16)  # [g, r]
        Pc = np.repeat(Pq[:, :, None], 4, axis=2)
    else:
        Pc = np.maximum(16, c4.max(axis=0))  # [g, r, c] exact max over cores
    if Pc.max() > 512:
        raise _CapacityOverflow(int(c4.max()))
    P = Pc.max(axis=2)  # [g, r] quad (PSUM/out) width
    rw = Pc.sum(axis=2)  # [g, r] xt row-band width
    W = rw.max(axis=1)  # [g] xt group width
    X = np.zeros(GROUPS, dtype=np.int64)
    np.cumsum(W[:-1], out=X[1:])
    Y = np.zeros(GROUPS * 4, dtype=np.int64)  # out offset per (g, r)
    np.cumsum(P.reshape(-1)[:-1], out=Y[1:])
    Y = Y.reshape(GROUPS, 4)
    xoff = np.zeros((GROUPS, 4, 4), dtype=np.int64)  # in-band col of slot c
    xoff[:, :, 1:] = np.cumsum(Pc, axis=2)[:, :, :-1]
    return (
        Pc.astype(np.int64),
        P.astype(np.int64),
        W.astype(np.int64),
        X,
        int(W.sum()),
        Y,
        int(P.sum()),
        xoff,
    )


def _build_bf16(Pc, P, W, X, TOTW, Y, TOTP, xoff) -> "bacc.Bacc":
    nc = bacc.Bacc("TRN2", target_bir_lowering=False, debug=False, num_devices=NCORES)
    xt = nc.declare_dram_parameter("xt", [128, TOTW], MM_DT, isOutput=False)
    wp = nc.declare_dram_parameter("wp", [128, GROUPS * 4 * O], MM_DT, isOutput=False)
    bp = nc.declare_dram_parameter("bp", [128, GROUPS * 4], F32, isOutput=False)
    ot = nc.declare_dram_parameter("ot", [128, TOTP], OT_DT, isOutput=True)

    with tile.TileContext(nc) as tc:
        with (
            tc.tile_pool(name="w", bufs=1) as w_pool,
            tc.tile_pool(name="xt", bufs=5) as xt_pool,
            tc.tile_pool(name="out", bufs=4) as out_pool,
            tc.tile_pool(name="psum", bufs=8, space="PSUM") as psum_pool,
        ):
            wp_t = w_pool.tile([128, GROUPS * 4 * O], MM_DT)
            nc.sync.dma_start(out=wp_t[:, : 4 * O], in_=wp[:, : 4 * O])
            bp_t = w_pool.tile([128, GROUPS * 4], F32)
            nc.gpsimd.dma_start(out=bp_t[:], in_=bp[:])
            wp2_queued = False

            # PE warm-up: throwaway matmuls on the already-loaded weight tile
            # keep the PE busy from ~3us so the clock ramp (HAM) completes
            # before the first real matmuls; their PSUM output is never read.
            warm = psum_pool.tile([128, 128], F32, space="PSUM", name="warm", tag="ps")
            for _ in range(14):
                nc.tensor.matmul(
                    out=warm[0:32, :],
                    lhsT=wp_t[0:32, 0:32],
                    rhs=wp_t[0:32, 0:128],
                    start=True,
                    stop=True,
                    tile_position=(0, 0),
                )

            for g in range(GROUPS):
                Wg, Xg = int(W[g]), int(X[g])
                OTg = int(P[g].sum())
                xt_t = xt_pool.tile([128, Wg], MM_DT, name="xt_t", tag="xt_t")
                if g == 0:
                    # row-split so quad (0, 0) can start as soon as possible
                    nc.sync.dma_start(out=xt_t[0:32, :], in_=xt[0:32, Xg : Xg + Wg])
                    nc.sync.dma_start(out=xt_t[32:128, :], in_=xt[32:128, Xg : Xg + Wg])
                else:
                    if not wp2_queued:
                        nc.sync.dma_start(out=wp_t[:, 4 * O :], in_=wp[:, 4 * O :])
                        wp2_queued = True
                    nc.sync.dma_start(out=xt_t[:], in_=xt[:, Xg : Xg + Wg])
                o_t = out_pool.tile([128, OTg], OT_DT, name="o_t", tag="o_t")

                ocol = 0
                for r in range(4):
                    Pgr = int(P[g, r])
                    psum = psum_pool.tile(
                        [128, Pgr], F32, space="PSUM", name="ps", tag="ps"
                    )
                    for c in range(4):
                        slot = 4 * g + c
                        Pgc = int(Pc[g, r, c])
                        x0 = int(xoff[g, r, c])
                        nc.tensor.matmul(
                            out=psum[32 * c : 32 * c + 32, :Pgc],
                            lhsT=wp_t[32 * r : 32 * r + 32, 32 * slot : 32 * slot + 32],
                            rhs=xt_t[32 * r : 32 * r + 32, x0 : x0 + Pgc],
                            start=True,
                            stop=True,
                            tile_position=(32 * r, 32 * c),
                        )
                    bias_ap = bp_t[:, 4 * g + r : 4 * g + r + 1]
                    if r % 2 == 0:
                        nc.vector.tensor_scalar_add(
                            o_t[:, ocol : ocol + Pgr], psum[:, :], bias_ap
                        )
                    else:
                        nc.scalar.activation(
                            o_t[:, ocol : ocol + Pgr],
                            psum[:, :],
                            mybir.ActivationFunctionType.Identity,
                            bias=bias_ap,
                            scale=1.0,
                        )
                    ocol += Pgr
                    if g == GROUPS - 1:
                        # drain the final group per quad across the DMA rings
                        engs = [nc.gpsimd, nc.scalar, nc.sync, nc.scalar]
                        Ygr = int(Y[g, r])
                        engs[r].dma_start(
                            out=ot[:, Ygr : Ygr + Pgr],
                            in_=o_t[:, ocol - Pgr : ocol],
                        )
                    elif r == 1:
                        Yg = int(Y[g, 0])
                        nc.gpsimd.dma_start(
                            out=ot[:, Yg : Yg + ocol], in_=o_t[:, :ocol]
                        )
                        half = ocol
                if g < GROUPS - 1:
                    Yg2 = int(Y[g, 2])
                    nc.scalar.dma_start(
                        out=ot[:, Yg2 : Yg2 + (OTg - half)], in_=o_t[:, half:]
                    )

    nc.compile()
    return nc


def _build_f32r(Pc, P, W, X, TOTW) -> "bacc.Bacc":
    nc = bacc.Bacc("TRN2", target_bir_lowering=False, debug=False, num_devices=NCORES)
    xt = nc.declare_dram_parameter("xt", [128, TOTW], MM_DT, isOutput=False)
    wp = nc.declare_dram_parameter("wp", [128, GROUPS * 4 * O], MM_DT, isOutput=False)
    bp = nc.declare_dram_parameter("bp", [O, E_LOCAL], F32, isOutput=False)
    ot = nc.declare_dram_parameter("ot", [128, TOTW], F32, isOutput=True)

    # spread ACT-assigned quads evenly through the schedule
    all_quads = [(g, r) for g in range(GROUPS) for r in range(4)]
    n_q = len(all_quads)
    act_set = (
        {
            all_quads[(i * n_q) // ACT_QUADS + ((n_q // ACT_QUADS) // 2)]
            for i in range(ACT_QUADS)
        }
        if ACT_QUADS
        else set()
    )
    quad_engine = {q: ("act" if q in act_set else "dve") for q in all_quads}

    with tile.TileContext(nc) as tc:
        with (
            tc.tile_pool(name="w", bufs=1) as w_pool,
            tc.tile_pool(name="xt", bufs=5) as xt_pool,
            tc.tile_pool(name="out", bufs=4) as out_pool,
            tc.tile_pool(name="psum", bufs=2, space="PSUM") as psum_pool,
        ):
            wp_t = w_pool.tile([128, GROUPS * 4 * O], MM_DT)
            nc.sync.dma_start(out=wp_t[:], in_=wp[:])
            bp_t = w_pool.tile([O, E_LOCAL], F32)
            nc.sync.dma_start(out=bp_t[:], in_=bp[:])

            for g in range(GROUPS):
                Wg, Xg = int(W[g]), int(X[g])
                xt_t = xt_pool.tile([128, Wg], MM_DT, name="xt_t", tag="xt_t")
                nc.sync.dma_start(out=xt_t[:], in_=xt[:, Xg : Xg + Wg])
                o_t = out_pool.tile([128, Wg], F32, name="o_t", tag="o_t")

                for r in range(4):
                    Pgr = int(P[g, r])
                    psum = psum_pool.tile(
                        [32, 2048], F32, space="PSUM", name="ps", tag="ps"
                    )
                    for c in range(4):
                        slot = 4 * g + c
                        nc.tensor.matmul(
                            out=psum[:, 512 * c : 512 * c + Pgr],
                            lhsT=wp_t[32 * r : 32 * r + 32, 32 * slot : 32 * slot + 32],
                            rhs=xt_t[32 * r : 32 * r + 32, c * Pgr : (c + 1) * Pgr],
                            start=True,
                            stop=True,
                            tile_position=(32 * r, 0),
                        )
                    eb = 16 * g + 4 * r  # first expert of the quad in bp
                    if quad_engine[(g, r)] == "dve":
                        psum_view = psum[:, :].rearrange("p (c t) -> p c t", c=4)[
                            :, :, :Pgr
                        ]
                        bias_view = bp_t[:, eb : eb + 4, None].to_broadcast([O, 4, Pgr])
                        out_view = o_t[32 * r : 32 * r + 32, : 4 * Pgr].rearrange(
                            "p (c t) -> p c t", c=4
                        )
                        nc.vector.tensor_tensor(
                            out=out_view,
                            in0=psum_view,
                            in1=bias_view,
                            op=mybir.AluOpType.add,
                        )
                    else:
                        for c in range(4):
                            nc.scalar.activation(
                                o_t[32 * r : 32 * r + 32, c * Pgr : (c + 1) * Pgr],
                                psum[:, 512 * c : 512 * c + Pgr],
                                mybir.ActivationFunctionType.Identity,
                                bias=bp_t[:, eb + c : eb + c + 1],
                                scale=1.0,
                            )
                nc.gpsimd.dma_start(out=ot[:, Xg : Xg + Wg], in_=o_t[:])

    nc.compile()
    return nc


def _pack(x, inds, w, b):
    """Host-side routing: sort tokens by expert, build per-core device arrays."""
    counts = np.bincount(inds, minlength=E)
    Pc, P, W, X, TOTW, Y, TOTP, xoff = _plan(counts)

    order = np.argsort(inds, kind="stable")
    sorted_inds = inds[order]
    starts = np.zeros(E, dtype=np.int64)
    np.cumsum(counts[:-1], out=starts[1:])
    slot = np.arange(N_TOK, dtype=np.int64) - starts[sorted_inds]

    e_all = np.arange(E)
    g_all = (e_all % E_LOCAL) // 16
    r_all = (e_all % 16) // 4
    c_all = e_all % 4
    xcol0 = X[g_all] + xoff[g_all, r_all, c_all]  # [E] xt start col per expert

    k_tok = sorted_inds // E_LOCAL
    r_tok = r_all[sorted_inds]
    xcol_tok = xcol0[sorted_inds] + slot

    mdt = mybir.dt.np(MM_DT)
    # xt_all[k, r, i, col] = x[token, i]
    xt_all = np.zeros((NCORES, 4, F, TOTW), dtype=mdt)
    xt_all[k_tok, r_tok, :, xcol_tok] = x[order].astype(mdt)
    xt = xt_all.reshape(NCORES, 128, TOTW)

    # wp[k, (r, i), (g, c, o)] = w[e, i, o] with e = 128k + 16g + 4r + c
    wp = np.ascontiguousarray(
        w.astype(mdt).reshape(NCORES, GROUPS, 4, 4, F, O).transpose(0, 2, 4, 1, 3, 5)
    ).reshape(NCORES, 128, GROUPS * 4 * O)

    b2 = b[:, 0, :]  # [E, O]
    if MM_DT == mybir.dt.float32r:
        bpk = np.ascontiguousarray(b2.reshape(NCORES, E_LOCAL, O).transpose(0, 2, 1))
    else:
        # bp[k, (c, o), (g, r)] = b[e, o]
        bpk = np.ascontiguousarray(
            b2.reshape(NCORES, GROUPS, 4, 4, O).transpose(0, 3, 4, 1, 2)
        ).reshape(NCORES, 128, GROUPS * 4)

    if MM_DT == mybir.dt.float32r:
        ocol_tok = xcol_tok
    else:
        ocol0 = Y[g_all, r_all]  # [E] out start col per expert (bf16 layout)
        ocol_tok = ocol0[sorted_inds] + slot

    plan = (Pc, P, W, X, TOTW, Y, TOTP, xoff)
    return plan, order, (k_tok, sorted_inds, ocol_tok), xt, wp, bpk


def _unpack(results, tok_addr, order):
    k_tok, sorted_inds, ocol_tok = tok_addr
    ot = np.stack([results[k]["ot"] for k in range(NCORES)])  # [k, 128, TOT*]
    out = np.empty((N_TOK, O), dtype=np.float32)
    if MM_DT == mybir.dt.float32r:
        r_tok = (sorted_inds % 16) // 4
        ot4 = ot.reshape(NCORES, 4, O, -1)  # [k, r, o, col]
        out[order] = ot4[k_tok, r_tok, :, ocol_tok]
    else:
        c_tok = sorted_inds % 4
        ot4 = ot.reshape(NCORES, 4, O, -1)  # [k, c, o, col]
        out[order] = ot4[k_tok, c_tok, :, ocol_tok]
    return out


def kernel(input, inds, w, b):
    x = np.ascontiguousarray(np.asarray(input, dtype=np.float32))
    inds = np.asarray(inds, dtype=np.int32)
    w = np.ascontiguousarray(np.asarray(w, dtype=np.float32))
    b = np.ascontiguousarray(np.asarray(b, dtype=np.float32))
    assert x.shape == (N_TOK, F) and inds.shape == (N_TOK,)
    assert w.shape == (E, F, O) and b.shape == (E, 1, O)

    try:
        plan, order, tok_addr, xt, wp, bpk = _pack(x, inds, w, b)
    except _CapacityOverflow:
        return (np.einsum("ni,nio->no", x, w[inds]) + b[inds, 0]).astype(np.float32)
    Pc, P, W, X, TOTW, Y, TOTP, xoff = plan

    key = (MM_DT, ACT_QUADS, Pc.tobytes())
    nc = _programs.get(key)
    if nc is None:
        if MM_DT == mybir.dt.float32r:
            nc = _build_f32r(Pc, P, W, X, TOTW)
        else:
            nc = _build_bf16(Pc, P, W, X, TOTW, Y, TOTP, xoff)
        _programs[key] = nc

    in_maps = [{"xt": xt[k], "wp": wp[k], "bp": bpk[k]} for k in range(NCORES)]
    res = run_bass_kernel_spmd(nc, in_maps, list(range(NCORES)))

    return _unpack(res.results, tok_addr, order)


def last_program():
    """The most recently compiled Bass program (for profiling in test.py)."""
    return next(iter(_programs.values())) if _programs else None

